# revision 1
# baseline (speedup 1.0000x reference)
"""Self-contained Trainium2 Bass kernel for the attention-like module:

    Q = x @ Wq.T + bq;  K = x @ Wk.T + bk;  V = x @ Wv.T + bv
    S = (Q.T @ K) / sqrt(dk);  A = softmax(S, axis=0);  out = V @ A

Sharding: data-parallel over the N=8192 rows across 8 NeuronCores; each core
computes its partial S_i = Q_i.T @ K_i, a 4MB f32 AllReduce sums them, each
core then applies the softmax and computes its row-shard of the output.

Layout choices (all picked so no on-chip transpose is ever needed):
  - host passes x.T and W.T (bf16), so projections produce Q,K in natural
    [row-part, col-free] layout and V in transposed [col-part, row-free] layout
  - S is computed in natural [i-part, j-free] layout; the softmax reduction
    over i (partitions) is done with a ones-vector matmul, and the resulting
    per-column 1/denominator is replicated across partitions with a rank-1
    matmul and multiplied in during the final PSUM eviction.
"""

import numpy as np
import ml_dtypes

N, D, NCORES, P, F = 8192, 1024, 8, 128, 512
NPC = N // NCORES   # rows per core
KC = D // P         # contraction chunks (8)
NCH = NPC // P      # row chunks per core (8)
JC = D // F         # 512-wide free chunks (2)
NORM = 1.0 / float(np.sqrt(D))

_cache = {}


def _build_nc():
    import concourse.mybir as mybir
    import concourse.tile as tile
    from concourse import bacc

    f32 = mybir.dt.float32
    bf16 = mybir.dt.bfloat16
    add = mybir.AluOpType.add
    mult = mybir.AluOpType.mult

    nc = bacc.Bacc("TRN2", target_bir_lowering=False, debug=False,
                   num_devices=NCORES)

    xT = nc.dram_tensor("xT", [D, NPC], bf16, kind="ExternalInput").ap()
    WqT = nc.dram_tensor("WqT", [D, D], bf16, kind="ExternalInput").ap()
    WkT = nc.dram_tensor("WkT", [D, D], bf16, kind="ExternalInput").ap()
    WvT = nc.dram_tensor("WvT", [D, D], bf16, kind="ExternalInput").ap()
    bqr = nc.dram_tensor("bqr", [P, D], f32, kind="ExternalInput").ap()
    bkr = nc.dram_tensor("bkr", [P, D], f32, kind="ExternalInput").ap()
    bvc = nc.dram_tensor("bvc", [P, KC], f32, kind="ExternalInput").ap()
    out = nc.dram_tensor("out", [NPC, D], f32, kind="ExternalOutput").ap()

    with tile.TileContext(nc) as tc:
        with tc.tile_pool(name="persist", bufs=1) as pp, \
             tc.tile_pool(name="stage", bufs=4) as sp, \
             tc.tile_pool(name="psA", bufs=6, space="PSUM") as psA, \
             tc.tile_pool(name="psB", bufs=2, space="PSUM") as psB, \
             tc.tile_pool(name="dram", bufs=1, space="DRAM") as dp:

            # ---- resident inputs (per-chunk DMAs so compute starts early) ----
            xt = pp.tile([P, KC, NPC], bf16, name="xt")
            xTr = xT.rearrange("(kc p) n -> p kc n", p=P)
            wq = pp.tile([P, KC, D], bf16, name="wq")
            wqr = WqT.rearrange("(kc p) j -> p kc j", p=P)
            wk = pp.tile([P, KC, D], bf16, name="wk")
            wkr = WkT.rearrange("(kc p) j -> p kc j", p=P)
            wv = pp.tile([P, KC, D], bf16, name="wv")
            wvr = WvT.rearrange("(kc p) j -> p kc j", p=P)
            for kc in range(KC):
                nc.sync.dma_start(xt[:, kc], xTr[:, kc])
                nc.sync.dma_start(wq[:, kc], wqr[:, kc])
                nc.sync.dma_start(wk[:, kc], wkr[:, kc])
            for kc in range(KC):
                nc.sync.dma_start(wv[:, kc], wvr[:, kc])
            bq_sb = pp.tile([P, D], f32, name="bq_sb")
            nc.sync.dma_start(bq_sb[:], bqr[:])
            bk_sb = pp.tile([P, D], f32, name="bk_sb")
            nc.sync.dma_start(bk_sb[:], bkr[:])
            bv_sb = pp.tile([P, KC], f32, name="bv_sb")
            nc.sync.dma_start(bv_sb[:], bvc[:])
            ones_b = pp.tile([P, 1], bf16, name="ones_b")
            nc.any.memset(ones_b[:], 1.0)
            ones_f = pp.tile([1, P], f32, name="ones_f")
            nc.any.memset(ones_f[:], 1.0)

            # ---- Q projection (full), then per-column-half pipeline ----
            q_sb = pp.tile([P, NCH, D], bf16, name="q_sb")
            k_sb = pp.tile([P, NCH, D], bf16, name="k_sb")
            for nch in range(NCH):
                for jc in range(JC):
                    ps = psA.tile([P, F], f32, tag="psA", name="ps_q")
                    for kc in range(KC):
                        nc.tensor.matmul(
                            ps[:],
                            xt[:, kc, nch * P:(nch + 1) * P],
                            wq[:, kc, jc * F:(jc + 1) * F],
                            start=(kc == 0), stop=(kc == KC - 1))
                    nc.vector.tensor_tensor(
                        q_sb[:, nch, jc * F:(jc + 1) * F],
                        ps[:], bq_sb[:, jc * F:(jc + 1) * F], add)

            # the scores matrix is split into two column halves; each half is
            # projected (K), contracted (S), and all-reduced independently so
            # the second half's compute hides under the first half's AllReduce
            s_bounce = [dp.tile([D, F], f32, name=f"s_bounce{h}")
                        for h in range(JC)]
            s_red = [dp.tile([D, F], f32, name=f"s_red{h}",
                             addr_space="Shared") for h in range(JC)]
            for h in range(JC):
                # K columns for this half
                for nch in range(NCH):
                    ps = psA.tile([P, F], f32, tag="psA", name="ps_k")
                    for kc in range(KC):
                        nc.tensor.matmul(
                            ps[:],
                            xt[:, kc, nch * P:(nch + 1) * P],
                            wk[:, kc, h * F:(h + 1) * F],
                            start=(kc == 0), stop=(kc == KC - 1))
                    nc.vector.tensor_tensor(
                        k_sb[:, nch, h * F:(h + 1) * F],
                        ps[:], bk_sb[:, h * F:(h + 1) * F], add)
                # partial scores for this half: [all qi, this j-half]
                for qch in range(KC):
                    st = sp.tile([P, F], f32, tag="sstage", name="st")
                    ps = psA.tile([P, F], f32, tag="psA", name="ps_s")
                    for nch in range(NCH):
                        nc.tensor.matmul(
                            ps[:],
                            q_sb[:, nch, qch * P:(qch + 1) * P],
                            k_sb[:, nch, h * F:(h + 1) * F],
                            start=(nch == 0), stop=(nch == NCH - 1))
                    nc.vector.tensor_copy(st[:], ps[:])
                    nc.sync.dma_start(
                        s_bounce[h][qch * P:(qch + 1) * P, :], st[:])
                nc.gpsimd.collective_compute(
                    "AllReduce", add,
                    replica_groups=[list(range(NCORES))],
                    ins=[s_bounce[h].opt()], outs=[s_red[h].opt()])

            # ---- V.T projection (independent of both AllReduces) ----
            vt_sb = pp.tile([P, KC, NPC], bf16, name="vt_sb")
            for ich in range(KC):
                for jc2 in range(NPC // F):
                    ps = psA.tile([P, F], f32, tag="psA", name="ps_v")
                    for kc in range(KC):
                        nc.tensor.matmul(
                            ps[:],
                            wv[:, kc, ich * P:(ich + 1) * P],
                            xt[:, kc, jc2 * F:(jc2 + 1) * F],
                            start=(kc == 0), stop=(kc == KC - 1))
                    nc.vector.tensor_scalar(
                        vt_sb[:, ich, jc2 * F:(jc2 + 1) * F],
                        ps[:], bv_sb[:, ich:ich + 1], None, add)

            # ---- per half: exp, denominator, output columns ----
            e_sb = pp.tile([P, KC, D], bf16, name="e_sb")
            den_sb = pp.tile([1, D], f32, name="den_sb")
            rden_sb = pp.tile([P, D], f32, name="rden_sb")
            for h in range(JC):
                for ich in range(KC):
                    s_t = sp.tile([P, F], f32, tag="sin", name="s_t")
                    nc.sync.dma_start(
                        s_t[:], s_red[h][ich * P:(ich + 1) * P, :])
                    nc.scalar.activation(
                        e_sb[:, ich, h * F:(h + 1) * F], s_t[:],
                        mybir.ActivationFunctionType.Exp, scale=NORM)
                psd = psB.tile([1, F], f32, tag="psB", name="psd")
                for ich in range(KC):
                    nc.tensor.matmul(
                        psd[:], ones_b[:, 0:1],
                        e_sb[:, ich, h * F:(h + 1) * F],
                        start=(ich == 0), stop=(ich == KC - 1))
                nc.vector.tensor_copy(den_sb[:, h * F:(h + 1) * F], psd[:])
                psr = psB.tile([P, F], f32, tag="psB", name="psr")
                nc.tensor.matmul(
                    psr[:], ones_f[:, 0:P],
                    den_sb[:, h * F:(h + 1) * F], start=True, stop=True)
                nc.vector.reciprocal(rden_sb[:, h * F:(h + 1) * F], psr[:])

                for nch in range(NCH):
                    ot = sp.tile([P, F], f32, tag="ostage", name="ot")
                    ps = psA.tile([P, F], f32, tag="psA", name="ps_o")
                    for ich in range(KC):
                        nc.tensor.matmul(
                            ps[:],
                            vt_sb[:, ich, nch * P:(nch + 1) * P],
                            e_sb[:, ich, h * F:(h + 1) * F],
                            start=(ich == 0), stop=(ich == KC - 1))
                    nc.vector.tensor_tensor(
                        ot[:], ps[:], rden_sb[:, h * F:(h + 1) * F], mult)
                    nc.sync.dma_start(
                        out[nch * P:(nch + 1) * P, h * F:(h + 1) * F], ot[:])

    nc.compile()
    return nc


def _prep_inputs(x, Wq, bq, Wk, bk, Wv, bv):
    bf16 = ml_dtypes.bfloat16
    xT_all = np.ascontiguousarray(np.asarray(x).astype(bf16).T)
    WqT = np.ascontiguousarray(np.asarray(Wq).astype(bf16).T)
    WkT = np.ascontiguousarray(np.asarray(Wk).astype(bf16).T)
    WvT = np.ascontiguousarray(np.asarray(Wv).astype(bf16).T)
    bqr = np.ascontiguousarray(
        np.broadcast_to(np.asarray(bq, np.float32), (P, D)))
    bkr = np.ascontiguousarray(
        np.broadcast_to(np.asarray(bk, np.float32), (P, D)))
    bvc = np.ascontiguousarray(
        np.asarray(bv, np.float32).reshape(KC, P).T)
    in_maps = []
    for c in range(NCORES):
        shard = np.ascontiguousarray(xT_all[:, c * NPC:(c + 1) * NPC])
        in_maps.append({
            "xT": shard, "WqT": WqT, "WkT": WkT, "WvT": WvT,
            "bqr": bqr, "bkr": bkr, "bvc": bvc,
        })
    return in_maps


def _ensure_axon_hooks_stub():
    # bass_utils imports antenv.axon_hooks when tracing is requested (also
    # via the BASS_TRACE env var); this image ships antenv without that
    # submodule, so install a no-op stub to degrade gracefully.
    import sys
    import types
    try:
        import antenv.axon_hooks  # noqa: F401
        return
    except ImportError:
        pass
    mod = types.ModuleType("antenv.axon_hooks")
    mod._hook = None
    mod.set_axon_ntff_profile_hook = lambda h: setattr(mod, "_hook", h)
    mod.get_axon_ntff_profile_hook = lambda: mod._hook
    sys.modules["antenv.axon_hooks"] = mod
    try:
        import antenv
        antenv.axon_hooks = mod
    except ImportError:
        pass


def kernel(x, Wq, bq, Wk, bk, Wv, bv, _trace=False):
    from concourse import bass_utils

    _ensure_axon_hooks_stub()

    if "nc" not in _cache:
        _cache["nc"] = _build_nc()
    nc = _cache["nc"]

    in_maps = _prep_inputs(x, Wq, bq, Wk, bk, Wv, bv)
    res = bass_utils.run_bass_kernel_spmd(
        nc, in_maps, core_ids=list(range(NCORES)), trace=_trace)
    _cache["last_result"] = res
    return np.concatenate(
        [res.results[c]["out"] for c in range(NCORES)], axis=0)

